# revision 10
# baseline (speedup 1.0000x reference)
"""Trainium2 Bass kernel for nn_Attention_29738353557815.

8-way tensor-parallel over heads:
  - core c owns q-heads {2c, 2c+1} and kv-head c//2 (k/v proj duplicated per core pair)
  - projections run weights-stationary off a host-pretransposed hidden^T, producing
    q/k in [head_dim, T] layout; v is produced transposed then PE-transposed back
  - rms-norm folded into ln/exp on ACT; rope tables (cos/sin * norm_w * sqrt(scale))
    are host-precomputed in [hd, T] layout; rotate-half done with partition-base
    shifted tensor_tensor ops
  - attention computed in S^T layout ([key, query] tiles): causal mask via
    gpsimd.affine_select, segment mask via scalar_tensor_tensor against iota;
    invalid (s,t) tiles are skipped entirely at build time (segment sparsity)
  - softmax denominator via ones-matmul column sums; normalization and sigmoid
    gating fused into one multiply before the o-projection
  - AllToAll (2 MiB/rank) redistributes gated attention so each core computes
    output rows [256c, 256c+256) with the full wo; host concatenates
"""
import sys

if "/opt/trn_rl_repo" not in sys.path:
    sys.path.insert(0, "/opt/trn_rl_repo")

import numpy as np

import concourse.bass as bass
from concourse import bacc
import concourse.mybir as mybir
import concourse.tile as tile
from concourse.bass_utils import run_bass_kernel_spmd
from concourse.masks import make_identity

F32 = mybir.dt.float32
F32R = mybir.dt.float32r
AF = mybir.ActivationFunctionType
OP = mybir.AluOpType

B, T, D = 1, 2048, 2048
NH, NKV, HD = 16, 4, 128
EPS = 1e-6
SCALE = HD ** -0.5
NCORES = 8
P = 128
NJ = T // 512      # 4 t-chunks of 512
NT = T // P        # 16 s-tiles of 128
DT = D // P        # 16 contraction tiles
TSL = T // NCORES  # 256 output rows per core

_program_cache: dict = {}


def _tile_flags(seg_end: np.ndarray):
    """Per (s-tile i, t-chunk j): (skip, needs_causal, needs_seg)."""
    flags = []
    for i in range(NT):
        smin, smax = P * i, P * i + P - 1
        se_lo = int(seg_end[smin])
        se_hi = int(seg_end[smax])
        row = []
        for j in range(NJ):
            t0, t1 = 512 * j, 512 * j + 511
            skip = (t1 < smin) or (t0 >= se_hi)
            causal = (not skip) and (t0 < smax)
            segm = (not skip) and (t1 >= se_lo)
            row.append((skip, causal, segm))
        flags.append(row)
    return tuple(tuple(r) for r in flags)


def _build_program(flags):
    nc = bacc.Bacc("TRN2", target_bir_lowering=False, debug=False,
                   num_devices=NCORES)

    hT_d = nc.dram_tensor("hT", [D, T], F32R, kind="ExternalInput")
    wqg_d = nc.dram_tensor("wqg", [D, 512], F32R, kind="ExternalInput")
    wkv_d = nc.dram_tensor("wkv", [D, 256], F32R, kind="ExternalInput")
    wo_d = nc.dram_tensor("wo", [D, D], F32R, kind="ExternalInput")
    tblq_d = nc.dram_tensor("tblq", [2, P, T], F32, kind="ExternalInput")
    tblk_d = nc.dram_tensor("tblk", [2, P, T], F32, kind="ExternalInput")
    iota_d = nc.dram_tensor("iota", [P, 512], F32, kind="ExternalInput")
    segrel_d = nc.dram_tensor("segrel", [P, NT, NJ], F32, kind="ExternalInput")
    out_d = nc.dram_tensor("out", [TSL, D], F32, kind="ExternalOutput")

    hT_re = hT_d.rearrange("(dt p) t -> p dt t", p=P)
    wqg_re = wqg_d.rearrange("(dt p) c -> p dt c", p=P)
    wkv_re = wkv_d.rearrange("(dt p) c -> p dt c", p=P)
    wo_re = wo_d.rearrange("(ht p) d -> p ht d", p=P)

    def r32(ap):
        return ap

    with tile.TileContext(nc) as tc:
        with (
            tc.tile_pool(name="consts", bufs=1) as consts,
            tc.tile_pool(name="perm", bufs=1) as perm,
            tc.tile_pool(name="hw", bufs=22) as hw,
            tc.tile_pool(name="tbl", bufs=8) as tblp,
            tc.tile_pool(name="tmp", bufs=6) as tmp,
            tc.tile_pool(name="ptp", bufs=3) as ptp,
            tc.tile_pool(name="ps", bufs=1, space="PSUM") as psp,
            tc.tile_pool(name="dram", bufs=1, space="DRAM") as dram,
        ):
            # ---- constants ----
            wqg_sb, wkv_sb = [], []
            for dt in range(DT):
                wq_t = consts.tile([P, 512], F32R, tag="wqg", bufs=DT,
                                   name=f"wqg{dt}")
                nc.sync.dma_start(wq_t[:], wqg_re[:, dt, :])
                wqg_sb.append(wq_t)
                wkv_t = consts.tile([P, 256], F32R, tag="wkv", bufs=DT,
                                    name=f"wkv{dt}")
                nc.sync.dma_start(wkv_t[:], wkv_re[:, dt, :])
                wkv_sb.append(wkv_t)
            iota_sb = consts.tile([P, 512], F32)
            nc.sync.dma_start(iota_sb[:], iota_d[:])
            segrel_sb = consts.tile([P, NT, NJ], F32)
            nc.sync.dma_start(segrel_sb[:], segrel_d[:])
            ones_f32 = consts.tile([P, P], F32)
            nc.vector.memset(ones_f32[:], 1.0)
            ones_sb = consts.tile([P, P], F32R)
            nc.vector.tensor_copy(ones_sb[:], ones_f32[:])
            ident_sb = consts.tile([P, P], F32)
            make_identity(nc, ident_sb[:])
            eps_sb = consts.tile([P, 1], F32)
            nc.vector.memset(eps_sb[:], EPS)

            # ---- persistent activations ----
            qTr = [perm.tile([P, T], F32R, tag=f"qTr{h}", name=f"qTr{h}") for h in range(2)]
            kTr = perm.tile([P, T], F32R, tag="kTr")
            gT = [perm.tile([P, T], F32, tag=f"gT{h}", name=f"gT{h}") for h in range(2)]
            v_sb = perm.tile([P, NT, P], F32R, tag="v_sb")

            a2a_in = dram.tile([NH * HD, TSL], F32R)
            a2a_out = dram.tile([NH * HD, TSL], F32R)

            # ================= phase B: projections =================
            for j in range(NJ):
                tsl = slice(512 * j, 512 * j + 512)
                hTt = []
                for dt in range(DT):
                    t_ = hw.tile([P, 512], F32R, tag="hw")
                    nc.sync.dma_start(t_[:], hT_re[:, dt, tsl])
                    hTt.append(t_)
                tb = {}
                for nm, dsrc, idx in (("cq", tblq_d, 0), ("sq", tblq_d, 1),
                                      ("ck", tblk_d, 0), ("sk", tblk_d, 1)):
                    t_ = tblp.tile([P, 512], F32, tag="tbl")
                    nc.sync.dma_start(t_[:], dsrc[idx, :, tsl])
                    tb[nm] = t_

                # c: 0=q0 1=q1 2=k 3=v 4=g0 5=g1
                for c in range(6):
                    if c < 2:
                        w_ap = lambda dt, c=c: wqg_sb[dt][:, 128 * c:128 * c + 128]
                    elif c == 2:
                        w_ap = lambda dt: wkv_sb[dt][:, 0:128]
                    elif c == 3:
                        w_ap = lambda dt: wkv_sb[dt][:, 128:256]
                    else:
                        w_ap = lambda dt, c=c: wqg_sb[dt][:, 256 + 128 * (c - 4):
                                                          384 + 128 * (c - 4)]

                    mm_ps = psp.tile([P, 512], F32, tag="mm", bufs=3)
                    for dt in range(DT):
                        nc.tensor.matmul(mm_ps[:], r32(w_ap(dt)), r32(hTt[dt][:]),
                                         start=(dt == 0), stop=(dt == DT - 1))

                    if c in (0, 1, 2):  # q0/q1/k: rms-norm + rope
                        dest = qTr[c][:, tsl] if c < 2 else kTr[:, tsl]
                        cosw = tb["cq"] if c < 2 else tb["ck"]
                        sinw = tb["sq"] if c < 2 else tb["sk"]
                        qpre = tmp.tile([P, 512], F32, tag="tmp")
                        nc.vector.tensor_copy(qpre[:], mm_ps[:])
                        q2 = tmp.tile([P, 512], F32R, tag="tmp2")
                        nc.scalar.activation(q2[:], mm_ps[:], AF.Square)
                        ssq_ps = psp.tile([P, 512], F32, tag="aux", bufs=2)
                        nc.tensor.matmul(ssq_ps[:], r32(ones_sb[:]), r32(q2[:]),
                                         start=True, stop=True)
                        rsv = tmp.tile([P, 512], F32, tag="tmp")
                        nc.scalar.activation(rsv[:], ssq_ps[:], AF.Ln,
                                             scale=1.0 / HD, bias=eps_sb[:, 0:1])
                        nc.scalar.activation(rsv[:], rsv[:], AF.Exp, scale=-0.5)
                        tcos = tmp.tile([P, 512], F32, tag="tmp")
                        nc.vector.tensor_tensor(tcos[:], qpre[:], cosw[:], OP.mult)
                        t2 = tmp.tile([P, 512], F32, tag="tmp")
                        # sinw table has halves pre-swapped host-side so both
                        # inputs share a base partition; only the output is shifted
                        nc.vector.tensor_tensor(t2[0:64, :], qpre[64:128, :],
                                                sinw[64:128, :], OP.mult)
                        nc.vector.tensor_tensor(t2[64:128, :], qpre[0:64, :],
                                                sinw[0:64, :], OP.mult)
                        nc.vector.tensor_tensor(t2[:], tcos[:], t2[:], OP.add)
                        nc.vector.tensor_tensor(dest, t2[:], rsv[:], OP.mult)
                    elif c in (4, 5):  # gate: raw copy
                        nc.vector.tensor_copy(gT[c - 4][:, tsl], mm_ps[:])
                    else:  # v: transpose [hd, t] -> [t, hd] tiles
                        vtmp = tmp.tile([P, 512], F32, tag="tmp")
                        nc.vector.tensor_copy(vtmp[:], mm_ps[:])
                        for kk in range(4):
                            tt = 4 * j + kk
                            trp = psp.tile([P, P], F32, tag="mm", bufs=3)
                            nc.tensor.transpose(trp[:], vtmp[:, 128 * kk:128 * kk + 128],
                                                ident_sb[:])
                            nc.vector.tensor_copy(v_sb[:, tt, :], trp[:])

            # ================= phase C: attention =================
            for h in range(2):
                for j in range(NJ):
                    tsl = slice(512 * j, 512 * j + 512)
                    valid = [i for i in range(NT) if not flags[i][j][0]]
                    ot_ps = psp.tile([P, 512], F32, tag="acc", bufs=3)
                    rs_ps = psp.tile([P, 512], F32, tag="acc", bufs=3)
                    last = len(valid) - 1
                    for idx, i in enumerate(valid):
                        _, needs_c, needs_s = flags[i][j]
                        st_ps = psp.tile([P, 512], F32, tag="mm", bufs=3)
                        nc.tensor.matmul(st_ps[:], r32(kTr[:, P * i:P * i + P]),
                                         r32(qTr[h][:, tsl]), start=True, stop=True)
                        pt = ptp.tile([P, 512], F32R, tag="pt")
                        nc.scalar.activation(pt[:], st_ps[:], AF.Exp)
                        if needs_c:
                            nc.gpsimd.affine_select(
                                out=pt[:], in_=pt[:], pattern=[[1, 512]],
                                compare_op=OP.is_ge, fill=0.0,
                                base=512 * j - P * i, channel_multiplier=-1)
                        if needs_s:
                            nc.vector.scalar_tensor_tensor(
                                out=pt[:], in0=iota_sb[:],
                                scalar=segrel_sb[:, i, j:j + 1], in1=pt[:],
                                op0=OP.is_lt, op1=OP.mult)
                        nc.tensor.matmul(ot_ps[:], r32(v_sb[:, i, :]), r32(pt[:]),
                                         start=(idx == 0), stop=(idx == last))
                        nc.tensor.matmul(rs_ps[:], r32(ones_sb[:]), r32(pt[:]),
                                         start=(idx == 0), stop=(idx == last))

                    # 1/rowsum and sigmoid gate (all on the exp/ln ACT table set)
                    rcp = tmp.tile([P, 512], F32, tag="tmp")
                    nc.scalar.activation(rcp[:], rs_ps[:], AF.Ln)
                    nc.scalar.activation(rcp[:], rcp[:], AF.Exp, scale=-1.0)
                    sg = tmp.tile([P, 512], F32, tag="tmp")
                    nc.scalar.activation(sg[:], gT[h][:, tsl], AF.Exp, scale=-1.0)
                    nc.scalar.activation(sg[:], sg[:], AF.Ln, bias=1.0)
                    nc.scalar.activation(sg[:], sg[:], AF.Exp, scale=-1.0)
                    nc.vector.tensor_tensor(sg[:], sg[:], rcp[:], OP.mult)
                    atg = tmp.tile([P, 512], F32R, tag="tmp2")
                    nc.vector.tensor_tensor(atg[:], ot_ps[:], sg[:], OP.mult)
                    # stage into a2a_in: shard s8 = 2j+half, rows 256*s8 + 128*h
                    for half in range(2):
                        s8 = 2 * j + half
                        nc.sync.dma_start(
                            a2a_in[256 * s8 + 128 * h: 256 * s8 + 128 * h + 128, :],
                            atg[:, 256 * half:256 * half + 256])

            # ================= phase D: A2A + o-proj =================
            nc.gpsimd.collective_compute(
                "AllToAll", OP.bypass,
                replica_groups=[list(range(NCORES))],
                ins=[a2a_in[:].opt()], outs=[a2a_out[:].opt()])
            a2a_out_re = a2a_out.rearrange("(ht p) t -> p ht t", p=P)
            ATall = []
            for ht in range(NT):
                at_t = perm.tile([P, TSL], F32R, tag="ATall", bufs=NT,
                                 name=f"ATall{ht}")
                nc.sync.dma_start(at_t[:], a2a_out_re[:, ht, :])
                ATall.append(at_t)

            for Dc in range(NJ):
                dsl = slice(512 * Dc, 512 * Dc + 512)
                ps0 = psp.tile([P, 512], F32, tag="acc", bufs=3)
                ps1 = psp.tile([P, 512], F32, tag="acc", bufs=3)
                for ht in range(NT):
                    wot = hw.tile([P, 512], F32R, tag="hw")
                    nc.sync.dma_start(wot[:], wo_re[:, ht, dsl])
                    nc.tensor.matmul(ps0[:], r32(ATall[ht][:, 0:128]), r32(wot[:]),
                                     start=(ht == 0), stop=(ht == NT - 1))
                    nc.tensor.matmul(ps1[:], r32(ATall[ht][:, 128:256]), r32(wot[:]),
                                     start=(ht == 0), stop=(ht == NT - 1))
                for m, ps_ in enumerate((ps0, ps1)):
                    o_sb = tmp.tile([P, 512], F32, tag="tmp")
                    nc.vector.tensor_copy(o_sb[:], ps_[:])
                    nc.sync.dma_start(out_d[128 * m:128 * m + 128, dsl], o_sb[:])

    nc.compile()
    return nc


def _host_prep(hidden_BTD, cos_BTK, sin_BTK, segment_ids_BT, position_ids_BT,
               wq, wk, wv, wo, q_norm_w, k_norm_w):
    hidden = np.ascontiguousarray(np.asarray(hidden_BTD, dtype=np.float32)[0])
    cos = np.asarray(cos_BTK, dtype=np.float32)[0]
    sin = np.asarray(sin_BTK, dtype=np.float32)[0]
    seg = np.asarray(segment_ids_BT)[0]
    pos = np.asarray(position_ids_BT)[0]
    wq = np.asarray(wq, dtype=np.float32)
    wk = np.asarray(wk, dtype=np.float32)
    wv = np.asarray(wv, dtype=np.float32)
    wo = np.ascontiguousarray(np.asarray(wo, dtype=np.float32))
    q_norm_w = np.asarray(q_norm_w, dtype=np.float32)
    k_norm_w = np.asarray(k_norm_w, dtype=np.float32)

    assert np.array_equal(pos, np.arange(T, dtype=pos.dtype)), \
        "kernel assumes position_ids == arange"
    assert np.all(np.diff(seg) >= 0), "kernel assumes sorted segment ids"

    hT = np.ascontiguousarray(hidden.T)
    sqrtS = np.float32(np.sqrt(SCALE))
    signv = np.where(np.arange(HD) < HD // 2, -1.0, 1.0).astype(np.float32)
    shuf = (np.arange(HD) + HD // 2) % HD

    def tables(w):
        cosw = (cos.T * w[:, None] * sqrtS).astype(np.float32)
        sinw = (sin.T * signv[:, None] * w[shuf][:, None] * sqrtS).astype(np.float32)
        sinswap = sinw[shuf]  # halves swapped: see rotate-half ops in _build_program
        return np.ascontiguousarray(np.stack([cosw, sinswap]))

    tblq = tables(q_norm_w)
    tblk = tables(k_norm_w)

    seg_end = np.searchsorted(seg, seg, side="right").astype(np.int64)
    iota = np.broadcast_to(np.arange(512, dtype=np.float32), (P, 512)).copy()
    segrel = np.zeros((P, NT, NJ), dtype=np.float32)
    for i in range(NT):
        for j in range(NJ):
            segrel[:, i, j] = seg_end[P * i:P * i + P] - 512.0 * j

    in_maps = []
    for c in range(NCORES):
        h0, h1 = 2 * c, 2 * c + 1
        g = c // 2
        wqg = np.ascontiguousarray(np.concatenate([
            wq[:, h0 * 256: h0 * 256 + 128],
            wq[:, h1 * 256: h1 * 256 + 128],
            wq[:, h0 * 256 + 128: h0 * 256 + 256],
            wq[:, h1 * 256 + 128: h1 * 256 + 256],
        ], axis=1))
        wkv = np.ascontiguousarray(np.concatenate([
            wk[:, g * 128:(g + 1) * 128], wv[:, g * 128:(g + 1) * 128]], axis=1))
        in_maps.append({
            "hT": hT, "wqg": wqg, "wkv": wkv, "wo": wo,
            "tblq": tblq, "tblk": tblk, "iota": iota, "segrel": segrel,
        })
    return in_maps, seg_end


def kernel(**inputs) -> np.ndarray:
    in_maps, seg_end = _host_prep(**inputs)
    flags = _tile_flags(seg_end)
    if flags not in _program_cache:
        _program_cache[flags] = _build_program(flags)
    nc = _program_cache[flags]
    res = run_bass_kernel_spmd(nc, in_maps, list(range(NCORES)))
    out = np.concatenate([res.results[c]["out"] for c in range(NCORES)], axis=0)
    return out[None].astype(np.float32)
